# revision 1
# baseline (speedup 1.0000x reference)
"""Multi-head(1) attention kernel for Trainium2, 8 NeuronCores SPMD.

Problem: x[4,4096,1024] @ {Wq,Wk,Wv}[1024,128] -> q,k,v; softmax(q k^T/sqrt(128)) v.

Sharding: core c -> (batch b = c//2, query-half h = c%2).
Each core receives xT = x[b].T (d_model on rows) with the 4096 columns permuted
so that "my" 2048 query rows come first. The core computes kT/v for all 4096
keys (key order is irrelevant under softmax), qT for its first 2048 columns,
and emits outT [128, 2048] = (attention output for its query rows)^T.

On-chip layouts (SBUF is [128 partitions x free]):
  qT, kT : [d_qk=128, seq]   (projection outputs, produced directly by PE)
  v      : [128-row chunk, d_v=128] chunks packed as [128, 4096]
  scoresT chunk: [k-chunk 128, q-block 512] = kT_chunk.T @ qT_block on PE
  U = exp(scoresT * 1/sqrt(dqk)) on ACT (no max subtraction: |scores| <~ 8)
  outT accum in PSUM over 32 k-chunks: out^T += v_chunk.T @ U_chunk
  denominators: DVE accumulates U chunks, PE ones-matmul does partition sum.

All tensors feeding matmuls are float32r (fp32 bits, PE streams at 1 cyc/row
for moving dim >= 256; the BIR verifier requires producers to emit f32r).

SBUF pools stay open for the whole kernel: letting phase-B pools reuse the
xt pool's addresses creates WAR waits against the 8 HWDGE queue semaphores
of the big xt DMAs, overflowing the per-instruction sync-wait limit.
PSUM pools are phase-scoped (only PE writes PSUM -> 1 wait).
"""

import math

import numpy as np

import concourse.bacc as bacc
import concourse.bass as bass
import concourse.mybir as mybir
from concourse.bass import ts
from concourse.masks import make_identity
from concourse.tile import TileContext

P = 128
D_MODEL = 1024
D_QK = 128
B = 4
S_FULL = 4096
N_CORES = 8

F32 = mybir.dt.float32
F32R = mybir.dt.float32r
AF = mybir.ActivationFunctionType

SM_SCALE = 1.0 / math.sqrt(D_QK)

MM_DT = F32R


def _mm(ap):
    return ap


def build_attention(nc: bass.Bass, S: int = S_FULL, SQ: int = S_FULL // 2, repeat: int = 1):
    """Emit the SPMD single-core program. S = #keys, SQ = #queries."""
    assert S % 512 == 0 and SQ % 512 == 0 and D_MODEL % P == 0
    DC = D_MODEL // P  # 8 d_model chunks
    NB = S // 512  # xT column blocks
    QNB = SQ // 512  # of which query blocks
    KC = S // P  # 32 k chunks

    xT = nc.dram_tensor("xT", [D_MODEL, S], MM_DT, kind="ExternalInput").ap()
    wq = nc.dram_tensor("Wq", [D_MODEL, D_QK], MM_DT, kind="ExternalInput").ap()
    bq = nc.dram_tensor("bq", [D_QK], F32, kind="ExternalInput").ap()
    wk = nc.dram_tensor("Wk", [D_MODEL, D_QK], MM_DT, kind="ExternalInput").ap()
    bk = nc.dram_tensor("bk", [D_QK], F32, kind="ExternalInput").ap()
    wv = nc.dram_tensor("Wv", [D_MODEL, D_QK], MM_DT, kind="ExternalInput").ap()
    bv = nc.dram_tensor("bv", [D_QK], F32, kind="ExternalInput").ap()
    outT = nc.dram_tensor("outT", [D_QK, SQ], F32, kind="ExternalOutput").ap()

    with TileContext(nc) as tc:
        lp = nc.allow_low_precision(reason="f32r accumulate of positive exp values")
        lp.__enter__()
        if repeat > 1:
            # benchmarking variant: run the whole kernel `repeat` times on
            # device so wall-clock timing can amortize the dispatch overhead
            loop_cm = tc.For_i(0, repeat, 1)
            loop_cm.__enter__()
        with (
            tc.tile_pool(name="persist", bufs=1) as pp,
            tc.tile_pool(name="xt_pool", bufs=2) as xp,
            tc.tile_pool(name="wka", bufs=3) as wka,
            tc.tile_pool(name="u_pool", bufs=6) as up,
            tc.tile_pool(name="dacc_pool", bufs=2) as dp,
            tc.tile_pool(name="wkb", bufs=3) as wkb,
            # po gets dedicated PSUM banks for the whole kernel: if its banks
            # were reused from phase-A pools, the accumulation-start matmul
            # would carry a bank-WAR wait on top of its RAW wait (2 > limit).
            tc.tile_pool(name="poB", bufs=2, space="PSUM") as poB,
            # one dedicated bank shared (sequentially) by the absorber scratch
            # and the per-q-block dps/bps epilogue tiles, for the same reason
            tc.tile_pool(name="misc", bufs=1, space="PSUM") as mpool,
        ):
            # --- constants ---
            w_sb = {}
            for nm, w in (("q", wq), ("k", wk), ("v", wv)):
                t = pp.tile([P, DC * D_QK], MM_DT, tag=f"w{nm}", name=f"w{nm}_sb")
                nc.sync.dma_start(
                    out=t.rearrange("p (c n) -> p c n", n=D_QK),
                    in_=w.rearrange("(c p) n -> p c n", p=P),
                )
                w_sb[nm] = t
            b_sb = {}
            for nm, b in (("q", bq), ("k", bk), ("v", bv)):
                t = pp.tile([P, 1], F32, tag=f"b{nm}", name=f"b{nm}_sb")
                nc.sync.dma_start(out=t, in_=b.unsqueeze(1))
                b_sb[nm] = t
            ident = pp.tile([P, P], F32, tag="ident")
            make_identity(nc, ident)
            ones_col = pp.tile([P, 1], F32, tag="ones_col")  # lhsT for partition-sum
            nc.gpsimd.memset(ones_col, 1.0)
            ones_col_r = pp.tile([P, 1], MM_DT, tag="ones_col_r")
            nc.vector.tensor_copy(out=ones_col_r, in_=ones_col)
            ones_row = pp.tile([1, P], F32, tag="ones_row")  # lhsT for bcast
            nc.gpsimd.memset(ones_row, 1.0)

            kT = pp.tile([P, S], MM_DT, tag="kT")
            vn = pp.tile([P, S], MM_DT, tag="vn")  # natural-v chunks side by side
            qT = pp.tile([P, SQ], MM_DT, tag="qT")

            # Wait-absorbers: an fp32r matmul lowers to an LDWEIGHTS+MATMUL
            # pair whose LW struct can encode only ONE sync wait. Tile happily
            # attaches 2+ (e.g. weight-DMA lane + xt-DMA lane), which walrus
            # rejects ("Too many sync wait commands"). Tiny PE matmuls reading
            # each DMA'd/POOL-produced tile make the PE observe those
            # semaphores first, so real matmuls need at most one wait.
            babs = wka.tile([P, 1], F32, tag="babs")

            # --- phase A: projections ---
            with (
                tc.tile_pool(name="psA", bufs=3, space="PSUM") as psA,
                tc.tile_pool(name="tpsA", bufs=2, space="PSUM") as tpsA,
            ):
                scr = mpool.tile([1, 1], F32, tag="misc")

                def pe_absorb(ap):
                    a = ap[:, 0:1]
                    if a.dtype != F32:
                        a = a.bitcast(F32)
                    nc.tensor.matmul(scr, a, a, start=True, stop=True)

                for nm in ("q", "k", "v"):
                    pe_absorb(w_sb[nm])  # weight DMA lanes
                    nc.scalar.copy(out=babs, in_=b_sb[nm])  # bias DMA lanes (ACT)
                # POOL-written tiles (each may be the last POOL tick after
                # scheduling, so absorb every one)
                pe_absorb(ident)
                pe_absorb(ones_col)
                pe_absorb(ones_row[0:1, 0:1].broadcast_to([1, 1]))

                for n in range(NB):
                    xt = xp.tile([P, DC * 512], MM_DT, tag="xt")
                    xt3 = xt.rearrange("p (c s) -> p c s", s=512)
                    xT3 = xT[:, ts(n, 512)].rearrange("(c p) s -> p c s", p=P)
                    hc = DC // 2
                    nc.sync.dma_start(out=xt3[:, :hc], in_=xT3[:, :hc])
                    nc.sync.dma_start(out=xt3[:, hc:], in_=xT3[:, hc:])
                    pe_absorb(xt)  # xt DMA lane
                    # kT block
                    kps = psA.tile([P, 512], F32, tag="ps")
                    for c in range(DC):
                        nc.tensor.matmul(
                            kps,
                            _mm(w_sb["k"][:, ts(c, D_QK)]),
                            _mm(xt[:, ts(c, 512)]),
                            start=(c == 0),
                            stop=(c == DC - 1),
                        )
                    nc.vector.tensor_scalar_add(
                        kT[:, ts(n, 512)], kps, b_sb["k"]
                    )
                    # v block: project to vT then PE-transpose to natural chunks
                    vps = psA.tile([P, 512], F32, tag="ps")
                    for c in range(DC):
                        nc.tensor.matmul(
                            vps,
                            _mm(w_sb["v"][:, ts(c, D_QK)]),
                            _mm(xt[:, ts(c, 512)]),
                            start=(c == 0),
                            stop=(c == DC - 1),
                        )
                    vt_tmp = wka.tile([P, 512], MM_DT, tag="vt_tmp")
                    nc.vector.tensor_scalar_add(vt_tmp, vps, b_sb["v"])
                    for j in range(4):
                        tps = tpsA.tile([P, P], F32, tag="tps")
                        nc.tensor.transpose(tps, vt_tmp[:, ts(j, P)].bitcast(F32), ident)
                        nc.scalar.copy(out=vn[:, ts(4 * n + j, P)], in_=tps)
                    # qT block (first SQ columns only)
                    if n < QNB:
                        qps = psA.tile([P, 512], F32, tag="ps")
                        for c in range(DC):
                            nc.tensor.matmul(
                                qps,
                                _mm(w_sb["q"][:, ts(c, D_QK)]),
                                _mm(xt[:, ts(c, 512)]),
                                start=(c == 0),
                                stop=(c == DC - 1),
                            )
                        nc.vector.tensor_scalar_add(
                            qT[:, ts(n, 512)], qps, b_sb["q"]
                        )

            # --- phase B: attention ---
            with tc.tile_pool(name="psB", bufs=2, space="PSUM") as psB:
                KP = KC // 2  # k-chunk pairs; one 1024-wide exp per pair
                for qb in range(QNB):
                    po = poB.tile([P, 512], F32, tag="po")
                    daccs = [
                        dp.tile([P, 512], MM_DT, tag=f"dacc{i}", name=f"dacc{i}_{qb}")
                        for i in range(2)
                    ]
                    dps = mpool.tile([1, 512], F32, tag="misc", name=f"dps_{qb}")
                    us: dict[int, object] = {}
                    # Software-pipelined over k-chunk PAIRS: two scores
                    # matmuls land in the two banks of one [128,1024] PSUM
                    # tile, a single wide exp (ACT fixed cost ~185ns/op is
                    # the phase-B limiter) produces u2, then two PV matmuls.
                    # Denominator: even chunk of each pair accumulates on
                    # DVE, odd chunk rides the PE (ones-matmul into dps).
                    for mp in range(KP + 1):
                        if mp < KP:
                            sps = psB.tile([P, 1024], F32, tag="ps")
                            for h in range(2):
                                nc.tensor.matmul(
                                    sps[:, ts(h, 512)],
                                    _mm(kT[:, ts(2 * mp + h, P)]),
                                    _mm(qT[:, ts(qb, 512)]),
                                    start=True,
                                    stop=True,
                                )
                            u = up.tile([P, 1024], MM_DT, tag="u")
                            nc.scalar.activation(u, sps, AF.Exp, scale=SM_SCALE)
                            us[mp] = u
                            nc.tensor.matmul(
                                dps,
                                _mm(ones_col_r),
                                u[:, ts(1, 512)],
                                start=(mp == 0),
                                stop=False,
                            )
                            if mp < 2:
                                nc.vector.tensor_copy(
                                    out=daccs[mp], in_=u[:, ts(0, 512)]
                                )
                            else:
                                nc.vector.tensor_add(
                                    out=daccs[mp % 2],
                                    in0=daccs[mp % 2],
                                    in1=u[:, ts(0, 512)],
                                )
                        if mp > 0:
                            u_prev = us.pop(mp - 1)
                            for h in range(2):
                                mm = 2 * (mp - 1) + h
                                nc.tensor.matmul(
                                    po,
                                    _mm(vn[:, ts(mm, P)]),
                                    _mm(u_prev[:, ts(h, 512)]),
                                    start=(mm == 0),
                                    stop=(mm == KC - 1),
                                )
                    nc.vector.tensor_add(out=daccs[0], in0=daccs[0], in1=daccs[1])
                    nc.tensor.matmul(
                        dps,
                        _mm(ones_col_r),
                        daccs[0],
                        start=False,
                        stop=True,
                    )
                    rec = wkb.tile([1, 512], F32, tag="rec")
                    nc.vector.reciprocal(out=rec, in_=dps)
                    bps = mpool.tile([P, 512], F32, tag="misc", name=f"bps_{qb}")
                    nc.tensor.matmul(bps, ones_row, rec, start=True, stop=True)
                    bsb = wkb.tile([P, 512], F32, tag="bsb")
                    nc.scalar.copy(out=bsb, in_=bps)
                    # evacuate po on ACT (not DVE): the next po-slot user's WAR
                    # then lands on the ACT sem it already waits on for u.
                    poc = wkb.tile([P, 512], F32, tag="poc")
                    nc.scalar.copy(out=poc, in_=po)
                    fin = wkb.tile([P, 512], F32, tag="fin")
                    nc.vector.tensor_mul(out=fin, in0=poc, in1=bsb)
                    nc.sync.dma_start(out=outT[:, ts(qb, 512)], in_=fin)

        if repeat > 1:
            loop_cm.__exit__(None, None, None)

    return nc


_NC_CACHE: dict = {}


def _get_nc(S: int = S_FULL, SQ: int = S_FULL // 2, repeat: int = 1):
    key = (S, SQ, repeat)
    if key not in _NC_CACHE:
        nc = bacc.Bacc("TRN2", debug=False)
        build_attention(nc, S, SQ, repeat)
        nc.compile()  # splits multi-waits into event semaphores (HW limit)
        _NC_CACHE[key] = nc
    return _NC_CACHE[key]


def make_in_maps(x, Wq, bq, Wk, bk, Wv, bv):
    """Per-core input dicts. Core c = (batch c//2, query-half c%2)."""
    x = np.asarray(x, dtype=np.float32)
    common = {
        "Wq": np.ascontiguousarray(Wq, dtype=np.float32),
        "bq": np.ascontiguousarray(bq, dtype=np.float32),
        "Wk": np.ascontiguousarray(Wk, dtype=np.float32),
        "bk": np.ascontiguousarray(bk, dtype=np.float32),
        "Wv": np.ascontiguousarray(Wv, dtype=np.float32),
        "bv": np.ascontiguousarray(bv, dtype=np.float32),
    }
    in_maps = []
    for c in range(N_CORES):
        b, h = divmod(c, 2)
        xb = x[b]  # [S, D]
        half = S_FULL // 2
        if h == 0:
            perm = xb
        else:
            perm = np.concatenate([xb[half:], xb[:half]], axis=0)
        in_maps.append({"xT": np.ascontiguousarray(perm.T), **common})
    return in_maps


def assemble_output(results):
    """results: list of 8 per-core dicts with 'outT' [128, 2048]."""
    half = S_FULL // 2
    out = np.empty((B, S_FULL, D_QK), dtype=np.float32)
    for c in range(N_CORES):
        b, h = divmod(c, 2)
        out[b, h * half : (h + 1) * half, :] = results[c]["outT"].T
    return out


def kernel(x, Wq, bq, Wk, bk, Wv, bv):
    from concourse.bass_utils import run_bass_kernel_spmd

    nc = _get_nc()
    in_maps = make_in_maps(x, Wq, bq, Wk, bk, Wv, bv)
    res = run_bass_kernel_spmd(nc, in_maps, list(range(N_CORES)))
    return assemble_output(res.results)

